# revision 28
# baseline (speedup 1.0000x reference)
"""ANI-style species-routed MLP (MoE routing) on 8 TRN2 NeuronCores.

Strategy:
- Data-parallel over molecules: core c handles molecules [128c, 128(c+1)).
- Host groups each core's 6144 atoms by species (counting sort), pads each
  species bucket to a shared uniform capacity, and ships the aev
  feature-major (transposed, partition-major, fp16) so features sit on SBUF
  partitions and every DMA is 128 long contiguous rows.
- Device computes, per species segment, the dense 4-layer MLP chain with
  fp16 matmuls (f32 PSUM accumulate), CELU via one exact trick:
      celu(x) + 0.1 = min(0.1*exp(10x), 0.1) + relu(x)
  The +0.1 offset is folded into the next layer's bias on the host
  (beta = b - 0.1 * rowsum(W)); the 0.1 output scale of the exp is folded
  into the activation bias (ln(0.1)).
- CELU needs no relu pass: u = max(z + (beta+0.1), min(e', 0.1)) exactly,
  by convexity of exp. Engine split: exp on ACT, min on GPSIMD (SBUF fp16),
  fused bias-add+max on DVE (the only extra PSUM read).
- Final per-molecule reduction on host (cheap), including the per-species
  output bias b4 - 0.1*rowsum(W4).
"""
import os
import sys

sys.path.insert(0, "/opt/trn_rl_repo")

from contextlib import ExitStack

import numpy as np

import concourse.bass as bass
import concourse.mybir as mybir
import concourse.tile as tile
from concourse import bacc
from concourse.bass_utils import run_bass_kernel_spmd

F32 = mybir.dt.float32
F16 = mybir.dt.float16
AF = mybir.ActivationFunctionType
ALU = mybir.AluOpType

B, A, F = 1024, 48, 384
S = 7
NCORES = 8
BM = B // NCORES  # molecules per core
ALPHA = 0.1
LN_ALPHA = float(np.log(ALPHA))

_CACHE = {}
LAST_EXEC_NS = None

# celu chunks with idx % MOD == PHASE use the relu-form (ACT-heavy):
#   u = stt(e, 0.1, r) = min(e,0.1)+relu(z+beta)   [ACT: exp+relu, DVE: stt]
# others use the max-form (DVE-heavy):
#   u = max(z+(beta+0.1), min(e,0.1))              [ACT: exp, DVE: min+combine]
RELU_FORM_SET = frozenset({0, 2, 4, 6})  # of idx % 8


def _build(cap):
    """SPMD graph: uniform per-species capacity `cap` (atoms). Matmul output
    tiles start at 512-column (PSUM bank) boundaries; the last half may be
    narrower than 512 so cap only needs 64-alignment."""
    assert cap % 64 == 0
    halves = [(o, min(512, cap - o)) for o in range(0, cap, 512)]
    zw = 512 * len(halves)  # bank-aligned z allocation width
    a_pad = S * cap
    nc = bacc.Bacc()

    xt_d = nc.declare_dram_parameter("xt", [128, S, 3, cap], F16, isOutput=False)
    w1_d = nc.declare_dram_parameter("w1t", [128, S, 3, 256], F16, isOutput=False)
    w2_d = nc.declare_dram_parameter("w2t", [128, S, 2, 192], F16, isOutput=False)
    w3_d = nc.declare_dram_parameter("w3t", [128, S, 2, 160], F16, isOutput=False)
    w4_d = nc.declare_dram_parameter("w4t", [128, S, 2, 1], F16, isOutput=False)
    # biases: [128, species, layer(3), kind(exp/comb/relu), chunk(2)]
    b_d = nc.declare_dram_parameter("biases", [128, S, 3, 3, 2], F32, isOutput=False)
    en_d = nc.declare_dram_parameter("energy", [1, a_pad], F32, isOutput=True)

    l1_m = [(0, 128), (128, 128)]
    l2_m = [(0, 128), (128, 64)]
    l3_m = [(0, 128), (128, 32)]
    l2_k = [(0, 128), (128, 128)]
    l3_k = [(0, 128), (128, 64)]
    l4_k = [(0, 128), (128, 32)]

    with tile.TileContext(nc) as tc, ExitStack() as ctx:
        wpool = ctx.enter_context(tc.tile_pool(name="weights", bufs=1))
        xpool = ctx.enter_context(tc.tile_pool(name="x", bufs=7))
        upool = ctx.enter_context(tc.tile_pool(name="u", bufs=4))
        tpool = ctx.enter_context(tc.tile_pool(name="t", bufs=6))
        zpool = ctx.enter_context(tc.tile_pool(name="z", bufs=3, space="PSUM"))
        z4pool = ctx.enter_context(tc.tile_pool(name="z4", bufs=2, space="PSUM"))
        epool = ctx.enter_context(tc.tile_pool(name="en", bufs=1))

        w1 = wpool.tile([128, S, 3, 256], F16)
        nc.sync.dma_start(w1[:], w1_d.ap())
        w2 = wpool.tile([128, S, 2, 192], F16)
        nc.sync.dma_start(w2[:], w2_d.ap())
        w3 = wpool.tile([128, S, 2, 160], F16)
        nc.sync.dma_start(w3[:], w3_d.ap())
        w4 = wpool.tile([128, S, 2, 1], F16)
        nc.sync.dma_start(w4[:], w4_d.ap())
        bb = wpool.tile([128, S, 3, 3, 2], F32)
        nc.sync.dma_start(bb[:], b_d.ap())

        en_sb = epool.tile([1, a_pad], F32)

        chunk_idx = 0

        def celu(z, u_out, s, l, m):
            nonlocal chunk_idx
            p = z.shape[0]
            n = z.shape[-1]
            bx = bb[:p, s, l - 1, 0, m : m + 1]
            bc = bb[:p, s, l - 1, 1, m : m + 1]
            br = bb[:p, s, l - 1, 2, m : m + 1]
            e = tpool.tile([128, cap], F16, tag="e")
            nc.scalar.activation(e[:p, :n], z[:], AF.Exp, bias=bx, scale=10.0)
            if chunk_idx % 8 in RELU_FORM_SET:
                r = tpool.tile([128, cap], F16, tag="r")
                nc.scalar.activation(r[:p, :n], z[:], AF.Relu, bias=br, scale=1.0)
                nc.vector.scalar_tensor_tensor(
                    u_out[:], e[:p, :n], ALPHA, r[:p, :n],
                    op0=ALU.min, op1=ALU.add,
                )
            else:
                mt = tpool.tile([128, cap], F16, tag="mt")
                nc.vector.tensor_scalar(
                    mt[:p, :n], e[:p, :n], ALPHA, None, op0=ALU.min
                )
                nc.vector.scalar_tensor_tensor(
                    u_out[:], z[:], bc, mt[:p, :n], op0=ALU.add, op1=ALU.max
                )
            chunk_idx += 1

        def emit_x(s):
            x = xpool.tile([128, 3, cap], F16, tag="x")
            nc.sync.dma_start(x[:], xt_d.ap()[:, s])
            return x

        def emit_l1(s, x):
            u1 = upool.tile([128, 2, cap], F16, tag="u1")
            for mi, (mo, mw) in enumerate(l1_m):
                z = zpool.tile([128, zw], F32, tag="z")
                for ho, hw in halves:
                    for k in range(3):
                        nc.tensor.matmul(
                            z[:mw, ho : ho + hw],
                            w1[:, s, k, mo : mo + mw],
                            x[:, k, ho : ho + hw],
                            start=(k == 0),
                            stop=(k == 2),
                        )
                celu(z[:mw, :cap], u1[:mw, mi, :], s, 1, mi)
            return u1

        def emit_mid(s, u_in, wgt, l, m_chunks, k_chunks):
            u_out = upool.tile([128, 2, cap], F16, tag=f"u{l}")
            for mi, (mo, mw) in enumerate(m_chunks):
                z = zpool.tile([128, zw], F32, tag="z")
                for ho, hw in halves:
                    for ki, (ko, kw) in enumerate(k_chunks):
                        nc.tensor.matmul(
                            z[:mw, ho : ho + hw],
                            wgt[:kw, s, ki, mo : mo + mw],
                            u_in[:kw, ki, ho : ho + hw],
                            start=(ki == 0),
                            stop=(ki == 1),
                        )
                celu(z[:mw, :cap], u_out[:mw, mi, :], s, l, mi)
            return u_out

        def emit_l4(s, u3):
            for ho, hw in halves:
                z4 = z4pool.tile([1, 512], F32, tag="z4")
                for ki, (ko, kw) in enumerate(l4_k):
                    nc.tensor.matmul(
                        z4[:, :hw],
                        w4[:kw, s, ki, 0:1],
                        u3[:kw, ki, ho : ho + hw],
                        start=(ki == 0),
                        stop=(ki == 1),
                    )
                oo = s * cap + ho
                nc.vector.tensor_copy(en_sb[0:1, oo : oo + hw], z4[:, :hw])

        pairs = [(0, 1, 2, 3), (4, 5, 6, None)]
        for pp in pairs:
            grp = [s for s in pp if s is not None]
            xs = {s: emit_x(s) for s in grp}
            u1s = {s: emit_l1(s, xs[s]) for s in grp}
            u2s = {s: emit_mid(s, u1s[s], w2, 2, l2_m, l2_k) for s in grp}
            u3s = {s: emit_mid(s, u2s[s], w3, 3, l3_m, l3_k) for s in grp}
            for s in grp:
                emit_l4(s, u3s[s])

        nc.sync.dma_start(en_d.ap(), en_sb[:])

    nc.compile()
    return nc


def _to_pmajor(wt, k_pad):
    """[S, M, K] weights -> [128, S, k_pad//128, M] fp16 partition-major."""
    s, m, k = wt.shape
    arr = np.zeros((s, m, k_pad), np.float32)
    arr[:, :, :k] = wt
    out = arr.transpose(2, 0, 1).reshape(k_pad // 128, 128, s, m).transpose(1, 2, 0, 3)
    return np.ascontiguousarray(out, dtype=np.float16)


def _prep_weights(W1, b1, W2, b2, W3, b3, W4, b4):
    beta1 = b1
    beta2 = b2 - ALPHA * W2.sum(axis=2)
    beta3 = b3 - ALPHA * W3.sum(axis=2)
    ec = (b4[:, 0] - ALPHA * W4[:, 0, :].sum(axis=1)).astype(np.float32)

    biases = np.zeros((S, 3, 3, 2, 128), np.float32)
    for li, beta in enumerate((beta1, beta2, beta3)):
        m = beta.shape[1]
        bx = np.zeros((S, 256), np.float32)
        bc = np.zeros((S, 256), np.float32)
        br = np.zeros((S, 256), np.float32)
        bx[:, :m] = 10.0 * beta + LN_ALPHA
        bc[:, :m] = beta + ALPHA
        br[:, :m] = beta
        biases[:, li, 0] = bx.reshape(S, 2, 128)
        biases[:, li, 1] = bc.reshape(S, 2, 128)
        biases[:, li, 2] = br.reshape(S, 2, 128)
    biases_p = np.ascontiguousarray(biases.transpose(4, 0, 1, 2, 3))

    return dict(
        w1t=_to_pmajor(W1, 384),
        w2t=_to_pmajor(W2, 256),
        w3t=_to_pmajor(W3, 256),
        w4t=_to_pmajor(W4, 256),
        biases=biases_p,
    ), ec


def kernel(species, aev, W1, b1, W2, b2, W3, b3, W4, b4):
    global LAST_EXEC_NS
    species = np.asarray(species)
    aev = np.asarray(aev, dtype=np.float32)
    args = [np.asarray(x, dtype=np.float32)
            for x in (W1, b1, W2, b2, W3, b3, W4, b4)]
    wp, ec = _prep_weights(*args)

    # --- host routing: per-core counting sort by species ---
    sp_c = species.reshape(NCORES, BM * A)
    counts = np.stack([np.bincount(sp_c[c], minlength=S) for c in range(NCORES)])
    cap = int(((counts.max() + 63) // 64) * 64)

    if cap not in _CACHE:
        _CACHE[cap] = _build(cap)
    nc = _CACHE[cap]

    aev_c = aev.reshape(NCORES, BM * A, F)
    in_maps = []
    perms = []
    for c in range(NCORES):
        perm = np.argsort(sp_c[c], kind="stable")
        perms.append(perm)
        xt = np.zeros((128, S, 3, cap), np.float16)
        pos = 0
        for s in range(S):
            n = counts[c, s]
            blk = aev_c[c][perm[pos : pos + n]].T.astype(np.float16)  # [384, n]
            xt[:, s, :, :n] = blk.reshape(3, 128, n).transpose(1, 0, 2)
            pos += n
        in_maps.append({"xt": xt, **wp})

    trace = bool(os.environ.get("KERNEL_TRACE"))
    res = run_bass_kernel_spmd(nc, in_maps, list(range(NCORES)), trace=trace)
    LAST_EXEC_NS = res.exec_time_ns

    # --- host reduction: scatter atom energies back to molecules ---
    out = np.zeros((NCORES, BM), np.float64)
    for c in range(NCORES):
        en = np.asarray(res.results[c]["energy"][0], np.float64)
        atom_e = np.empty(BM * A, np.float64)
        pos = 0
        for s in range(S):
            n = counts[c, s]
            atom_e[perms[c][pos : pos + n]] = en[s * cap : s * cap + n]
            pos += n
        out[c] = atom_e.reshape(BM, A).sum(axis=1)
        out[c] += np.asarray(ec, np.float64)[sp_c[c]].reshape(BM, A).sum(axis=1)
    return out.reshape(B).astype(np.float32)


# revision 39
# speedup vs baseline: 1.0485x; 1.0485x over previous
"""ANI-style species-routed MLP (MoE routing) on 8 TRN2 NeuronCores.

Strategy:
- Data-parallel over molecules: core c handles molecules [128c, 128(c+1)).
- Host groups each core's 6144 atoms by species (counting sort), pads each
  species bucket to a shared uniform capacity, and ships the aev
  feature-major (transposed, partition-major, fp16) so features sit on SBUF
  partitions and every DMA is 128 long contiguous rows.
- Device computes, per species segment, the dense 4-layer MLP chain with
  fp16 matmuls (f32 PSUM accumulate), CELU via one exact trick:
      celu(x) + 0.1 = min(0.1*exp(10x), 0.1) + relu(x)
  The +0.1 offset is folded into the next layer's bias on the host
  (beta = b - 0.1 * rowsum(W)); the 0.1 output scale of the exp is folded
  into the activation bias (ln(0.1)).
- CELU needs no relu pass: u = max(z + (beta+0.1), min(e', 0.1)) exactly,
  by convexity of exp. Engine split: exp on ACT, min on GPSIMD (SBUF fp16),
  fused bias-add+max on DVE (the only extra PSUM read).
- Final per-molecule reduction on host (cheap), including the per-species
  output bias b4 - 0.1*rowsum(W4).
"""
import os
import sys

sys.path.insert(0, "/opt/trn_rl_repo")

from contextlib import ExitStack

import numpy as np

import concourse.bass as bass
import concourse.mybir as mybir
import concourse.tile as tile
from concourse import bacc
from concourse.bass_utils import run_bass_kernel_spmd

F32 = mybir.dt.float32
F16 = mybir.dt.float16
AF = mybir.ActivationFunctionType
ALU = mybir.AluOpType

B, A, F = 1024, 48, 384
S = 7
NCORES = 8
BM = B // NCORES  # molecules per core
ALPHA = 0.1
LN_ALPHA = float(np.log(ALPHA))

_CACHE = {}
LAST_EXEC_NS = None

# build-variant flags (A/B testable in one process; cache key includes them)
WARMUP_MM = False
W1_SPLIT = True
EN_SPLIT = False
GROUP_MODE = "g43"  # g43 | g2 | g3 | wave
UBUFS = 4
TBUFS = 6
RELU_MOD = 8
PACK3 = False  # pack L3 m1 (32-row) chunks of 3 species into one celu pass
PACKS3 = ((0, 1, 2), (3,), (4, 5, 6))

# celu chunks with idx % MOD == PHASE use the relu-form (ACT-heavy):
#   u = stt(e, 0.1, r) = min(e,0.1)+relu(z+beta)   [ACT: exp+relu, DVE: stt]
# others use the max-form (DVE-heavy):
#   u = max(z+(beta+0.1), min(e,0.1))              [ACT: exp, DVE: min+combine]
RELU_FORM_SET = frozenset({0, 2, 4, 6})  # of idx % 8


def _build(cap):
    """SPMD graph: uniform per-species capacity `cap` (atoms). Matmul output
    tiles start at 512-column (PSUM bank) boundaries; the last half may be
    narrower than 512 so cap only needs 64-alignment."""
    assert cap % 64 == 0
    halves = [(o, min(512, cap - o)) for o in range(0, cap, 512)]
    zw = 512 * len(halves)  # bank-aligned z allocation width
    a_pad = S * cap
    nc = bacc.Bacc()

    xt_d = nc.declare_dram_parameter("xt", [128, S, 3, cap], F16, isOutput=False)
    w1_d = nc.declare_dram_parameter("w1t", [128, S, 3, 256], F16, isOutput=False)
    w2_d = nc.declare_dram_parameter("w2t", [128, S, 2, 192], F16, isOutput=False)
    w3_d = nc.declare_dram_parameter("w3t", [128, S, 2, 160], F16, isOutput=False)
    w4_d = nc.declare_dram_parameter("w4t", [128, S, 2, 1], F16, isOutput=False)
    # biases: [128, species, layer(3), kind(exp/comb/relu), chunk(2)]
    b_d = nc.declare_dram_parameter("biases", [128, S, 3, 3, 2], F32, isOutput=False)
    bpk3_d = nc.declare_dram_parameter("bpk3", [128, len(PACKS3), 3], F32,
                                       isOutput=False)
    w4pk_d = nc.declare_dram_parameter("w4pk", [128, S, 1], F16, isOutput=False)
    en_d = nc.declare_dram_parameter("energy", [1, a_pad], F32, isOutput=True)

    l1_m = [(0, 128), (128, 128)]
    l2_m = [(0, 128), (128, 64)]
    l3_m = [(0, 128), (128, 32)]
    l2_k = [(0, 128), (128, 128)]
    l3_k = [(0, 128), (128, 64)]
    l4_k = [(0, 128), (128, 32)]

    with tile.TileContext(nc) as tc, ExitStack() as ctx:
        wpool = ctx.enter_context(tc.tile_pool(name="weights", bufs=1))
        xpool = ctx.enter_context(tc.tile_pool(name="x", bufs=7))
        upool = ctx.enter_context(tc.tile_pool(name="u", bufs=UBUFS))
        tpool = ctx.enter_context(tc.tile_pool(name="t", bufs=TBUFS))
        zpool = ctx.enter_context(tc.tile_pool(name="z", bufs=3, space="PSUM"))
        z4pool = ctx.enter_context(tc.tile_pool(name="z4", bufs=2, space="PSUM"))
        epool = ctx.enter_context(tc.tile_pool(name="en", bufs=1))

        w1 = wpool.tile([128, S, 3, 256], F16)
        w2 = wpool.tile([128, S, 2, 192], F16)
        w3 = wpool.tile([128, S, 2, 160], F16)
        w4 = wpool.tile([128, S, 2, 1], F16)
        bb = wpool.tile([128, S, 3, 3, 2], F32)
        bpk3 = wpool.tile([128, len(PACKS3), 3], F32)
        nc.sync.dma_start(bpk3[:], bpk3_d.ap())
        w4pk = wpool.tile([128, S, 1], F16)
        nc.sync.dma_start(w4pk[:], w4pk_d.ap())
        if W1_SPLIT:
            x_tiles = {}
            for ws in range(S):
                xt_tile = xpool.tile([128, 3, cap], F16, tag="x")
                x_tiles[ws] = xt_tile
                nc.sync.dma_start(x_tiles[ws][:], xt_d.ap()[:, ws])
                nc.sync.dma_start(w1[:, ws], w1_d.ap()[:, ws])
                nc.sync.dma_start(w2[:, ws], w2_d.ap()[:, ws])
                nc.sync.dma_start(w3[:, ws], w3_d.ap()[:, ws])
                nc.sync.dma_start(w4[:, ws], w4_d.ap()[:, ws])
                if ws == 0:
                    nc.sync.dma_start(bb[:], b_d.ap())
        else:
            x_tiles = None
            nc.sync.dma_start(w1[:], w1_d.ap())
            nc.sync.dma_start(w2[:], w2_d.ap())
            nc.sync.dma_start(w3[:], w3_d.ap())
            nc.sync.dma_start(w4[:], w4_d.ap())
            nc.sync.dma_start(bb[:], b_d.ap())

        en_sb = epool.tile([1, a_pad], F32)

        if WARMUP_MM:
            scrap = wpool.tile([128, 512], F16, tag="scrap")
            nc.vector.memset(scrap[:], 0.0)
            for wi in range(WARMUP_MM if WARMUP_MM is not True else 10):
                zs = zpool.tile([128, zw], F32, tag="z")
                for wj in range(3):
                    nc.tensor.matmul(zs[:, 0:512], scrap[:, 0:128], scrap[:],
                                     start=(wj == 0), stop=(wj == 2))

        chunk_idx = 0

        def celu_b(z, u_out, bx, bc, br):
            nonlocal chunk_idx
            p = z.shape[0]
            n = z.shape[-1]
            e = tpool.tile([128, cap], F16, tag="e")
            nc.scalar.activation(e[:p, :n], z[:], AF.Exp, bias=bx, scale=10.0)
            if chunk_idx % RELU_MOD in RELU_FORM_SET:
                r = tpool.tile([128, cap], F16, tag="r")
                nc.scalar.activation(r[:p, :n], z[:], AF.Relu, bias=br, scale=1.0)
                nc.vector.scalar_tensor_tensor(
                    u_out[:], e[:p, :n], ALPHA, r[:p, :n],
                    op0=ALU.min, op1=ALU.add,
                )
            else:
                mt = tpool.tile([128, cap], F16, tag="mt")
                nc.vector.tensor_scalar(
                    mt[:p, :n], e[:p, :n], ALPHA, None, op0=ALU.min
                )
                nc.vector.scalar_tensor_tensor(
                    u_out[:], z[:], bc, mt[:p, :n], op0=ALU.add, op1=ALU.max
                )
            chunk_idx += 1

        def celu(z, u_out, s, l, m):
            p = z.shape[0]
            celu_b(z, u_out,
                   bb[:p, s, l - 1, 0, m : m + 1],
                   bb[:p, s, l - 1, 1, m : m + 1],
                   bb[:p, s, l - 1, 2, m : m + 1])

        def emit_x(s):
            if x_tiles is not None:
                return x_tiles[s]
            x = xpool.tile([128, 3, cap], F16, tag="x")
            nc.sync.dma_start(x[:], xt_d.ap()[:, s])
            return x

        def emit_l1(s, x):
            u1 = upool.tile([128, 2, cap], F16, tag="u1")
            for mi, (mo, mw) in enumerate(l1_m):
                z = zpool.tile([128, zw], F32, tag="z")
                for ho, hw in halves:
                    for k in range(3):
                        nc.tensor.matmul(
                            z[:mw, ho : ho + hw],
                            w1[:, s, k, mo : mo + mw],
                            x[:, k, ho : ho + hw],
                            start=(k == 0),
                            stop=(k == 2),
                        )
                celu(z[:mw, :cap], u1[:mw, mi, :], s, 1, mi)
            return u1

        def emit_mid(s, u_in, wgt, l, m_chunks, k_chunks):
            u_out = upool.tile([128, 2, cap], F16, tag=f"u{l}")
            for mi, (mo, mw) in enumerate(m_chunks):
                z = zpool.tile([128, zw], F32, tag="z")
                for ho, hw in halves:
                    for ki, (ko, kw) in enumerate(k_chunks):
                        nc.tensor.matmul(
                            z[:mw, ho : ho + hw],
                            wgt[:kw, s, ki, mo : mo + mw],
                            u_in[:kw, ki, ho : ho + hw],
                            start=(ki == 0),
                            stop=(ki == 1),
                        )
                celu(z[:mw, :cap], u_out[:mw, mi, :], s, l, mi)
            return u_out

        def emit_l3_m0(s, u2):
            u3m0 = upool.tile([128, cap], F16, tag="u3")
            z = zpool.tile([128, zw], F32, tag="z")
            for ho, hw in halves:
                for ki, (ko, kw) in enumerate(l3_k):
                    nc.tensor.matmul(
                        z[:, ho : ho + hw],
                        w3[:kw, s, ki, 0:128],
                        u2[:kw, ki, ho : ho + hw],
                        start=(ki == 0),
                        stop=(ki == 1),
                    )
            celu(z[:, :cap], u3m0[:], s, 3, 0)
            return u3m0

        def emit_l3_pack(pk, u2s_d):
            np_ = 32 * len(pk)
            u3pk = upool.tile([128, cap], F16, tag="u3pk")
            z = zpool.tile([128, zw], F32, tag="z")
            for j, s in enumerate(pk):
                bp = 32 * j
                for ho, hw in halves:
                    for ki, (ko, kw) in enumerate(l3_k):
                        nc.tensor.matmul(
                            z[bp : bp + 32, ho : ho + hw],
                            w3[:kw, s, ki, 128:160],
                            u2s_d[s][:kw, ki, ho : ho + hw],
                            start=(ki == 0),
                            stop=(ki == 1),
                        )
            pi = PACKS3.index(pk)
            celu_b(z[:np_, :cap], u3pk[:np_, :],
                   bpk3[:np_, pi, 0:1], bpk3[:np_, pi, 1:2], bpk3[:np_, pi, 2:3])
            return u3pk

        def emit_l4(s, u3):
            for ho, hw in halves:
                z4 = z4pool.tile([1, 512], F32, tag="z4")
                for ki, (ko, kw) in enumerate(l4_k):
                    if PACK3 and ki == 1:
                        u3m0, u3pk, bp = u3
                        rhs = u3pk[bp : bp + 32, ho : ho + hw]
                        lhs = w4pk[bp : bp + 32, s, 0:1]
                    elif PACK3:
                        u3m0, u3pk, bp = u3
                        rhs = u3m0[:kw, ho : ho + hw]
                        lhs = w4[:kw, s, 0, 0:1]
                    else:
                        rhs = u3[:kw, ki, ho : ho + hw]
                        lhs = w4[:kw, s, ki, 0:1]
                    nc.tensor.matmul(
                        z4[:, :hw],
                        lhs,
                        rhs,
                        start=(ki == 0),
                        stop=(ki == 1),
                    )
                oo = s * cap + ho
                nc.vector.tensor_copy(en_sb[0:1, oo : oo + hw], z4[:, :hw])

        if GROUP_MODE == "wave":
            xs = {s: emit_x(s) for s in range(S)}
            u1s, u2s, u3s = {}, {}, {}
            for w in range(S + 3):
                for s in range(S):
                    st = w - s
                    if st == 0:
                        u1s[s] = emit_l1(s, xs[s])
                    elif st == 1:
                        u2s[s] = emit_mid(s, u1s.pop(s), w2, 2, l2_m, l2_k)
                    elif st == 2:
                        u3s[s] = emit_mid(s, u2s.pop(s), w3, 3, l3_m, l3_k)
                    elif st == 3:
                        emit_l4(s, u3s.pop(s))
        else:
            pairs = {"g43": [(0, 1, 2, 3), (4, 5, 6)],
                     "g2": [(0, 1), (2, 3), (4, 5), (6,)],
                     "g3": [(0, 1, 2), (3, 4, 5), (6,)]}[GROUP_MODE]
            for grp in pairs:
                xs = {s: emit_x(s) for s in grp}
                u1s = {s: emit_l1(s, xs[s]) for s in grp}
                u2s = {s: emit_mid(s, u1s[s], w2, 2, l2_m, l2_k) for s in grp}
                if PACK3:
                    u3s = {}
                    m0s = {s: emit_l3_m0(s, u2s[s]) for s in grp}
                    for pk in PACKS3:
                        if pk[0] not in grp:
                            continue
                        u3pk = emit_l3_pack(pk, u2s)
                        for j, s in enumerate(pk):
                            u3s[s] = (m0s[s], u3pk, 32 * j)
                else:
                    u3s = {s: emit_mid(s, u2s[s], w3, 3, l3_m, l3_k)
                           for s in grp}
                for s in grp:
                    emit_l4(s, u3s[s])
                if EN_SPLIT:
                    lo = min(grp) * cap
                    hi = (max(grp) + 1) * cap
                    nc.sync.dma_start(en_d.ap()[:, lo:hi], en_sb[0:1, lo:hi])
        if not EN_SPLIT:
            nc.sync.dma_start(en_d.ap(), en_sb[:])

    nc.compile()
    return nc


def _to_pmajor(wt, k_pad):
    """[S, M, K] weights -> [128, S, k_pad//128, M] fp16 partition-major."""
    s, m, k = wt.shape
    arr = np.zeros((s, m, k_pad), np.float32)
    arr[:, :, :k] = wt
    out = arr.transpose(2, 0, 1).reshape(k_pad // 128, 128, s, m).transpose(1, 2, 0, 3)
    return np.ascontiguousarray(out, dtype=np.float16)


def _prep_weights(W1, b1, W2, b2, W3, b3, W4, b4):
    beta1 = b1
    beta2 = b2 - ALPHA * W2.sum(axis=2)
    beta3 = b3 - ALPHA * W3.sum(axis=2)
    ec = (b4[:, 0] - ALPHA * W4[:, 0, :].sum(axis=1)).astype(np.float32)

    biases = np.zeros((S, 3, 3, 2, 128), np.float32)
    for li, beta in enumerate((beta1, beta2, beta3)):
        m = beta.shape[1]
        bx = np.zeros((S, 256), np.float32)
        bc = np.zeros((S, 256), np.float32)
        br = np.zeros((S, 256), np.float32)
        bx[:, :m] = 10.0 * beta + LN_ALPHA
        bc[:, :m] = beta + ALPHA
        br[:, :m] = beta
        biases[:, li, 0] = bx.reshape(S, 2, 128)
        biases[:, li, 1] = bc.reshape(S, 2, 128)
        biases[:, li, 2] = br.reshape(S, 2, 128)
    biases_p = np.ascontiguousarray(biases.transpose(4, 0, 1, 2, 3))
    bpk3 = np.zeros((128, len(PACKS3), 3), np.float32)
    w4pk_a = np.zeros((128, S, 1), np.float16)
    for pi, pk in enumerate(PACKS3):
        for j, sp in enumerate(pk):
            bt = beta3[sp][128:160]
            bpk3[32 * j : 32 * j + 32, pi, 0] = 10.0 * bt + LN_ALPHA
            bpk3[32 * j : 32 * j + 32, pi, 1] = bt + ALPHA
            bpk3[32 * j : 32 * j + 32, pi, 2] = bt
            w4pk_a[32 * j : 32 * j + 32, sp, 0] = W4[sp, 0, 128:160]

    return dict(
        w1t=_to_pmajor(W1, 384),
        w2t=_to_pmajor(W2, 256),
        w3t=_to_pmajor(W3, 256),
        w4t=_to_pmajor(W4, 256),
        biases=biases_p, bpk3=bpk3, w4pk=w4pk_a,
    ), ec


def kernel(species, aev, W1, b1, W2, b2, W3, b3, W4, b4):
    global LAST_EXEC_NS
    species = np.asarray(species)
    aev = np.asarray(aev, dtype=np.float32)
    args = [np.asarray(x, dtype=np.float32)
            for x in (W1, b1, W2, b2, W3, b3, W4, b4)]
    wp, ec = _prep_weights(*args)

    # --- host routing: per-core counting sort by species ---
    sp_c = species.reshape(NCORES, BM * A)
    counts = np.stack([np.bincount(sp_c[c], minlength=S) for c in range(NCORES)])
    cap = int(((counts.max() + 63) // 64) * 64)

    key = (cap, WARMUP_MM, W1_SPLIT, EN_SPLIT, RELU_FORM_SET, GROUP_MODE, UBUFS, TBUFS, RELU_MOD, PACK3)
    if key not in _CACHE:
        _CACHE[key] = _build(cap)
    nc = _CACHE[key]

    aev_c = aev.reshape(NCORES, BM * A, F)
    in_maps = []
    perms = []
    for c in range(NCORES):
        perm = np.argsort(sp_c[c], kind="stable")
        perms.append(perm)
        xt = np.zeros((128, S, 3, cap), np.float16)
        pos = 0
        for s in range(S):
            n = counts[c, s]
            blk = aev_c[c][perm[pos : pos + n]].T.astype(np.float16)  # [384, n]
            xt[:, s, :, :n] = blk.reshape(3, 128, n).transpose(1, 0, 2)
            pos += n
        in_maps.append({"xt": xt, **wp})

    trace = bool(os.environ.get("KERNEL_TRACE"))
    res = run_bass_kernel_spmd(nc, in_maps, list(range(NCORES)), trace=trace)
    LAST_EXEC_NS = res.exec_time_ns

    # --- host reduction: scatter atom energies back to molecules ---
    out = np.zeros((NCORES, BM), np.float64)
    for c in range(NCORES):
        en = np.asarray(res.results[c]["energy"][0], np.float64)
        atom_e = np.empty(BM * A, np.float64)
        pos = 0
        for s in range(S):
            n = counts[c, s]
            atom_e[perms[c][pos : pos + n]] = en[s * cap : s * cap + n]
            pos += n
        out[c] = atom_e.reshape(BM, A).sum(axis=1)
        out[c] += np.asarray(ec, np.float64)[sp_c[c]].reshape(BM, A).sum(axis=1)
    return out.reshape(B).astype(np.float32)


# revision 40
# speedup vs baseline: 1.2531x; 1.1952x over previous
"""ANI-style species-routed MLP (MoE routing) on 8 TRN2 NeuronCores.

Strategy:
- Data-parallel over molecules: core c handles molecules [128c, 128(c+1)).
- Host groups each core's 6144 atoms by species (counting sort), pads each
  species bucket to a shared uniform capacity, and ships the aev
  feature-major (transposed, partition-major, fp16) so features sit on SBUF
  partitions and every DMA is 128 long contiguous rows.
- Device computes, per species segment, the dense 4-layer MLP chain with
  fp16 matmuls (f32 PSUM accumulate), CELU via one exact trick:
      celu(x) + 0.1 = min(0.1*exp(10x), 0.1) + relu(x)
  The +0.1 offset is folded into the next layer's bias on the host
  (beta = b - 0.1 * rowsum(W)); the 0.1 output scale of the exp is folded
  into the activation bias (ln(0.1)).
- CELU needs no relu pass: u = max(z + (beta+0.1), min(e', 0.1)) exactly,
  by convexity of exp. Engine split: exp on ACT, min on GPSIMD (SBUF fp16),
  fused bias-add+max on DVE (the only extra PSUM read).
- Final per-molecule reduction on host (cheap), including the per-species
  output bias b4 - 0.1*rowsum(W4).
"""
import os
import sys

sys.path.insert(0, "/opt/trn_rl_repo")

from contextlib import ExitStack

import numpy as np

import concourse.bass as bass
import concourse.mybir as mybir
import concourse.tile as tile
from concourse import bacc
from concourse.bass_utils import run_bass_kernel_spmd

F32 = mybir.dt.float32
F16 = mybir.dt.float16
AF = mybir.ActivationFunctionType
ALU = mybir.AluOpType

B, A, F = 1024, 48, 384
S = 7
NCORES = 8
BM = B // NCORES  # molecules per core
ALPHA = 0.1
LN_ALPHA = float(np.log(ALPHA))

_CACHE = {}
LAST_EXEC_NS = None

# build-variant flags (A/B testable in one process; cache key includes them)
WARMUP_MM = False
W1_SPLIT = True
EN_SPLIT = False
GROUP_MODE = "g43"  # g43 | g2 | g3 | wave
UBUFS = 4
TBUFS = 6
RELU_MOD = 8
PACK3 = False  # pack L3 m1 (32-row) chunks of 3 species into one celu pass
PACKS3 = ((0, 1, 2), (3,), (4, 5, 6))

# celu chunks with idx % MOD == PHASE use the relu-form (ACT-heavy):
#   u = stt(e, 0.1, r) = min(e,0.1)+relu(z+beta)   [ACT: exp+relu, DVE: stt]
# others use the max-form (DVE-heavy):
#   u = max(z+(beta+0.1), min(e,0.1))              [ACT: exp, DVE: min+combine]
RELU_FORM_SET = frozenset({0, 2, 4, 6})  # of idx % 8


def _build(cap):
    """SPMD graph: uniform per-species capacity `cap` (atoms). Matmul output
    tiles start at 512-column (PSUM bank) boundaries; the last half may be
    narrower than 512 so cap only needs 64-alignment."""
    assert cap % 64 == 0
    halves = [(o, min(512, cap - o)) for o in range(0, cap, 512)]
    zw = 512 * len(halves)  # bank-aligned z allocation width
    a_pad = S * cap
    nc = bacc.Bacc()

    xt_d = nc.declare_dram_parameter("xt", [128, S, 3, cap], F16, isOutput=False)
    w1_d = nc.declare_dram_parameter("w1t", [128, S, 3, 256], F16, isOutput=False)
    w2_d = nc.declare_dram_parameter("w2t", [128, S, 2, 192], F16, isOutput=False)
    w3_d = nc.declare_dram_parameter("w3t", [128, S, 2, 160], F16, isOutput=False)
    w4_d = nc.declare_dram_parameter("w4t", [128, S, 2, 1], F16, isOutput=False)
    # biases: [128, species, layer(3), kind(exp/comb/relu), chunk(2)]
    b_d = nc.declare_dram_parameter("biases", [128, S, 3, 3, 2], F32, isOutput=False)
    bpk3_d = nc.declare_dram_parameter("bpk3", [128, len(PACKS3), 3], F32,
                                       isOutput=False)
    w4pk_d = nc.declare_dram_parameter("w4pk", [128, S, 1], F16, isOutput=False)
    en_d = nc.declare_dram_parameter("energy", [1, a_pad], F32, isOutput=True)

    l1_m = [(0, 128), (128, 128)]
    l2_m = [(0, 128), (128, 64)]
    l3_m = [(0, 128), (128, 32)]
    l2_k = [(0, 128), (128, 128)]
    l3_k = [(0, 128), (128, 64)]
    l4_k = [(0, 128), (128, 32)]

    with tile.TileContext(nc) as tc, ExitStack() as ctx:
        wpool = ctx.enter_context(tc.tile_pool(name="weights", bufs=1))
        xpool = ctx.enter_context(tc.tile_pool(name="x", bufs=7))
        upool = ctx.enter_context(tc.tile_pool(name="u", bufs=UBUFS))
        tpool = ctx.enter_context(tc.tile_pool(name="t", bufs=TBUFS))
        zpool = ctx.enter_context(tc.tile_pool(name="z", bufs=3, space="PSUM"))
        z4pool = ctx.enter_context(tc.tile_pool(name="z4", bufs=2, space="PSUM"))
        epool = ctx.enter_context(tc.tile_pool(name="en", bufs=1))

        w1 = wpool.tile([128, S, 3, 256], F16)
        w2 = wpool.tile([128, S, 2, 192], F16)
        w3 = wpool.tile([128, S, 2, 160], F16)
        w4 = wpool.tile([128, S, 2, 1], F16)
        bb = wpool.tile([128, S, 3, 3, 2], F32)
        bpk3 = wpool.tile([128, len(PACKS3), 3], F32)
        nc.sync.dma_start(bpk3[:], bpk3_d.ap())
        w4pk = wpool.tile([128, S, 1], F16)
        nc.sync.dma_start(w4pk[:], w4pk_d.ap())
        if W1_SPLIT:
            x_tiles = {}
            for ws in range(S):
                xt_tile = xpool.tile([128, 3, cap], F16, tag="x")
                x_tiles[ws] = xt_tile
                nc.sync.dma_start(x_tiles[ws][:], xt_d.ap()[:, ws])
                nc.sync.dma_start(w1[:, ws], w1_d.ap()[:, ws])
                nc.sync.dma_start(w2[:, ws], w2_d.ap()[:, ws])
                nc.sync.dma_start(w3[:, ws], w3_d.ap()[:, ws])
                nc.sync.dma_start(w4[:, ws], w4_d.ap()[:, ws])
                if ws == 0:
                    nc.sync.dma_start(bb[:], b_d.ap())
        else:
            x_tiles = None
            nc.sync.dma_start(w1[:], w1_d.ap())
            nc.sync.dma_start(w2[:], w2_d.ap())
            nc.sync.dma_start(w3[:], w3_d.ap())
            nc.sync.dma_start(w4[:], w4_d.ap())
            nc.sync.dma_start(bb[:], b_d.ap())

        en_sb = epool.tile([1, a_pad], F32)

        if WARMUP_MM:
            scrap = wpool.tile([128, 512], F16, tag="scrap")
            nc.vector.memset(scrap[:], 0.0)
            for wi in range(WARMUP_MM if WARMUP_MM is not True else 10):
                zs = zpool.tile([128, zw], F32, tag="z")
                for wj in range(3):
                    nc.tensor.matmul(zs[:, 0:512], scrap[:, 0:128], scrap[:],
                                     start=(wj == 0), stop=(wj == 2))

        chunk_idx = 0

        def celu_b(z, u_out, bx, bc, br):
            nonlocal chunk_idx
            p = z.shape[0]
            n = z.shape[-1]
            e = tpool.tile([128, cap], F16, tag="e")
            nc.scalar.activation(e[:p, :n], z[:], AF.Exp, bias=bx, scale=10.0)
            if chunk_idx % RELU_MOD in RELU_FORM_SET:
                r = tpool.tile([128, cap], F16, tag="r")
                nc.scalar.activation(r[:p, :n], z[:], AF.Relu, bias=br, scale=1.0)
                nc.vector.scalar_tensor_tensor(
                    u_out[:], e[:p, :n], ALPHA, r[:p, :n],
                    op0=ALU.min, op1=ALU.add,
                )
            else:
                mt = tpool.tile([128, cap], F16, tag="mt")
                nc.vector.tensor_scalar(
                    mt[:p, :n], e[:p, :n], ALPHA, None, op0=ALU.min
                )
                nc.vector.scalar_tensor_tensor(
                    u_out[:], z[:], bc, mt[:p, :n], op0=ALU.add, op1=ALU.max
                )
            chunk_idx += 1

        def celu(z, u_out, s, l, m):
            p = z.shape[0]
            celu_b(z, u_out,
                   bb[:p, s, l - 1, 0, m : m + 1],
                   bb[:p, s, l - 1, 1, m : m + 1],
                   bb[:p, s, l - 1, 2, m : m + 1])

        def emit_x(s):
            if x_tiles is not None:
                return x_tiles[s]
            x = xpool.tile([128, 3, cap], F16, tag="x")
            nc.sync.dma_start(x[:], xt_d.ap()[:, s])
            return x

        def emit_l1(s, x, fine=False):
            tiles_n = ([(o, min(256, cap - o)) for o in range(0, cap, 256)]
                       if fine else halves)
            u1 = upool.tile([128, 2, cap], F16, tag="u1")
            for mi, (mo, mw) in enumerate(l1_m):
                z = zpool.tile([128, zw], F32, tag="z")
                for ho, hw in tiles_n:
                    for k in range(3):
                        nc.tensor.matmul(
                            z[:mw, ho : ho + hw],
                            w1[:, s, k, mo : mo + mw],
                            x[:, k, ho : ho + hw],
                            start=(k == 0),
                            stop=(k == 2),
                        )
                celu(z[:mw, :cap], u1[:mw, mi, :], s, 1, mi)
            return u1

        def emit_mid(s, u_in, wgt, l, m_chunks, k_chunks):
            u_out = upool.tile([128, 2, cap], F16, tag=f"u{l}")
            for mi, (mo, mw) in enumerate(m_chunks):
                z = zpool.tile([128, zw], F32, tag="z")
                for ho, hw in halves:
                    for ki, (ko, kw) in enumerate(k_chunks):
                        nc.tensor.matmul(
                            z[:mw, ho : ho + hw],
                            wgt[:kw, s, ki, mo : mo + mw],
                            u_in[:kw, ki, ho : ho + hw],
                            start=(ki == 0),
                            stop=(ki == 1),
                        )
                celu(z[:mw, :cap], u_out[:mw, mi, :], s, l, mi)
            return u_out

        def emit_l3_m0(s, u2):
            u3m0 = upool.tile([128, cap], F16, tag="u3")
            z = zpool.tile([128, zw], F32, tag="z")
            for ho, hw in halves:
                for ki, (ko, kw) in enumerate(l3_k):
                    nc.tensor.matmul(
                        z[:, ho : ho + hw],
                        w3[:kw, s, ki, 0:128],
                        u2[:kw, ki, ho : ho + hw],
                        start=(ki == 0),
                        stop=(ki == 1),
                    )
            celu(z[:, :cap], u3m0[:], s, 3, 0)
            return u3m0

        def emit_l3_pack(pk, u2s_d):
            np_ = 32 * len(pk)
            u3pk = upool.tile([128, cap], F16, tag="u3pk")
            z = zpool.tile([128, zw], F32, tag="z")
            for j, s in enumerate(pk):
                bp = 32 * j
                for ho, hw in halves:
                    for ki, (ko, kw) in enumerate(l3_k):
                        nc.tensor.matmul(
                            z[bp : bp + 32, ho : ho + hw],
                            w3[:kw, s, ki, 128:160],
                            u2s_d[s][:kw, ki, ho : ho + hw],
                            start=(ki == 0),
                            stop=(ki == 1),
                        )
            pi = PACKS3.index(pk)
            celu_b(z[:np_, :cap], u3pk[:np_, :],
                   bpk3[:np_, pi, 0:1], bpk3[:np_, pi, 1:2], bpk3[:np_, pi, 2:3])
            return u3pk

        def emit_l4(s, u3):
            for ho, hw in halves:
                z4 = z4pool.tile([1, 512], F32, tag="z4")
                for ki, (ko, kw) in enumerate(l4_k):
                    if PACK3 and ki == 1:
                        u3m0, u3pk, bp = u3
                        rhs = u3pk[bp : bp + 32, ho : ho + hw]
                        lhs = w4pk[bp : bp + 32, s, 0:1]
                    elif PACK3:
                        u3m0, u3pk, bp = u3
                        rhs = u3m0[:kw, ho : ho + hw]
                        lhs = w4[:kw, s, 0, 0:1]
                    else:
                        rhs = u3[:kw, ki, ho : ho + hw]
                        lhs = w4[:kw, s, ki, 0:1]
                    nc.tensor.matmul(
                        z4[:, :hw],
                        lhs,
                        rhs,
                        start=(ki == 0),
                        stop=(ki == 1),
                    )
                oo = s * cap + ho
                nc.vector.tensor_copy(en_sb[0:1, oo : oo + hw], z4[:, :hw])

        if GROUP_MODE == "wave":
            xs = {s: emit_x(s) for s in range(S)}
            u1s, u2s, u3s = {}, {}, {}
            for w in range(S + 3):
                for s in range(S):
                    st = w - s
                    if st == 0:
                        u1s[s] = emit_l1(s, xs[s])
                    elif st == 1:
                        u2s[s] = emit_mid(s, u1s.pop(s), w2, 2, l2_m, l2_k)
                    elif st == 2:
                        u3s[s] = emit_mid(s, u2s.pop(s), w3, 3, l3_m, l3_k)
                    elif st == 3:
                        emit_l4(s, u3s.pop(s))
        else:
            pairs = {"g43": [(0, 1, 2, 3), (4, 5, 6)],
                     "g2": [(0, 1), (2, 3), (4, 5), (6,)],
                     "g3": [(0, 1, 2), (3, 4, 5), (6,)]}[GROUP_MODE]
            for grp in pairs:
                xs = {s: emit_x(s) for s in grp}
                u1s = {s: emit_l1(s, xs[s], fine=(s == 0)) for s in grp}
                u2s = {s: emit_mid(s, u1s[s], w2, 2, l2_m, l2_k) for s in grp}
                if PACK3:
                    u3s = {}
                    m0s = {s: emit_l3_m0(s, u2s[s]) for s in grp}
                    for pk in PACKS3:
                        if pk[0] not in grp:
                            continue
                        u3pk = emit_l3_pack(pk, u2s)
                        for j, s in enumerate(pk):
                            u3s[s] = (m0s[s], u3pk, 32 * j)
                    for s in grp:
                        emit_l4(s, u3s[s])
                        if EN_SPLIT:
                            lo = s * cap
                            nc.sync.dma_start(en_d.ap()[:, lo : lo + cap],
                                              en_sb[0:1, lo : lo + cap])
                else:
                    for s in grp:
                        u3 = emit_mid(s, u2s[s], w3, 3, l3_m, l3_k)
                        emit_l4(s, u3)
                        if EN_SPLIT:
                            lo = s * cap
                            nc.sync.dma_start(en_d.ap()[:, lo : lo + cap],
                                              en_sb[0:1, lo : lo + cap])
        if not EN_SPLIT:
            nc.sync.dma_start(en_d.ap(), en_sb[:])

    nc.compile()
    return nc


def _to_pmajor(wt, k_pad):
    """[S, M, K] weights -> [128, S, k_pad//128, M] fp16 partition-major."""
    s, m, k = wt.shape
    arr = np.zeros((s, m, k_pad), np.float32)
    arr[:, :, :k] = wt
    out = arr.transpose(2, 0, 1).reshape(k_pad // 128, 128, s, m).transpose(1, 2, 0, 3)
    return np.ascontiguousarray(out, dtype=np.float16)


def _prep_weights(W1, b1, W2, b2, W3, b3, W4, b4):
    beta1 = b1
    beta2 = b2 - ALPHA * W2.sum(axis=2)
    beta3 = b3 - ALPHA * W3.sum(axis=2)
    ec = (b4[:, 0] - ALPHA * W4[:, 0, :].sum(axis=1)).astype(np.float32)

    biases = np.zeros((S, 3, 3, 2, 128), np.float32)
    for li, beta in enumerate((beta1, beta2, beta3)):
        m = beta.shape[1]
        bx = np.zeros((S, 256), np.float32)
        bc = np.zeros((S, 256), np.float32)
        br = np.zeros((S, 256), np.float32)
        bx[:, :m] = 10.0 * beta + LN_ALPHA
        bc[:, :m] = beta + ALPHA
        br[:, :m] = beta
        biases[:, li, 0] = bx.reshape(S, 2, 128)
        biases[:, li, 1] = bc.reshape(S, 2, 128)
        biases[:, li, 2] = br.reshape(S, 2, 128)
    biases_p = np.ascontiguousarray(biases.transpose(4, 0, 1, 2, 3))
    bpk3 = np.zeros((128, len(PACKS3), 3), np.float32)
    w4pk_a = np.zeros((128, S, 1), np.float16)
    for pi, pk in enumerate(PACKS3):
        for j, sp in enumerate(pk):
            bt = beta3[sp][128:160]
            bpk3[32 * j : 32 * j + 32, pi, 0] = 10.0 * bt + LN_ALPHA
            bpk3[32 * j : 32 * j + 32, pi, 1] = bt + ALPHA
            bpk3[32 * j : 32 * j + 32, pi, 2] = bt
            w4pk_a[32 * j : 32 * j + 32, sp, 0] = W4[sp, 0, 128:160]

    return dict(
        w1t=_to_pmajor(W1, 384),
        w2t=_to_pmajor(W2, 256),
        w3t=_to_pmajor(W3, 256),
        w4t=_to_pmajor(W4, 256),
        biases=biases_p, bpk3=bpk3, w4pk=w4pk_a,
    ), ec


def kernel(species, aev, W1, b1, W2, b2, W3, b3, W4, b4):
    global LAST_EXEC_NS
    species = np.asarray(species)
    aev = np.asarray(aev, dtype=np.float32)
    args = [np.asarray(x, dtype=np.float32)
            for x in (W1, b1, W2, b2, W3, b3, W4, b4)]
    wp, ec = _prep_weights(*args)

    # --- host routing: per-core counting sort by species ---
    sp_c = species.reshape(NCORES, BM * A)
    counts = np.stack([np.bincount(sp_c[c], minlength=S) for c in range(NCORES)])
    cap = int(((counts.max() + 63) // 64) * 64)

    key = (cap, WARMUP_MM, W1_SPLIT, EN_SPLIT, RELU_FORM_SET, GROUP_MODE, UBUFS, TBUFS, RELU_MOD, PACK3)
    if key not in _CACHE:
        _CACHE[key] = _build(cap)
    nc = _CACHE[key]

    aev_c = aev.reshape(NCORES, BM * A, F)
    in_maps = []
    perms = []
    for c in range(NCORES):
        perm = np.argsort(sp_c[c], kind="stable")
        perms.append(perm)
        xt = np.zeros((128, S, 3, cap), np.float16)
        pos = 0
        for s in range(S):
            n = counts[c, s]
            blk = aev_c[c][perm[pos : pos + n]].T.astype(np.float16)  # [384, n]
            xt[:, s, :, :n] = blk.reshape(3, 128, n).transpose(1, 0, 2)
            pos += n
        in_maps.append({"xt": xt, **wp})

    trace = bool(os.environ.get("KERNEL_TRACE"))
    res = run_bass_kernel_spmd(nc, in_maps, list(range(NCORES)), trace=trace)
    LAST_EXEC_NS = res.exec_time_ns

    # --- host reduction: scatter atom energies back to molecules ---
    out = np.zeros((NCORES, BM), np.float64)
    for c in range(NCORES):
        en = np.asarray(res.results[c]["energy"][0], np.float64)
        atom_e = np.empty(BM * A, np.float64)
        pos = 0
        for s in range(S):
            n = counts[c, s]
            atom_e[perms[c][pos : pos + n]] = en[s * cap : s * cap + n]
            pos += n
        out[c] = atom_e.reshape(BM, A).sum(axis=1)
        out[c] += np.asarray(ec, np.float64)[sp_c[c]].reshape(BM, A).sum(axis=1)
    return out.reshape(B).astype(np.float32)
